# revision 7
# baseline (speedup 1.0000x reference)
"""Trainium2 Bass kernel for nn_BasicBlock (binary-conv residual block).

Math (reference):
  h  = BN3( RPReLU1(BN1(bconv(sign(x), w1))) + x )
  out= BN4( RPReLU2(BN2(bconv(sign(h), w2))) + h )
with training-mode BN over the FULL batch (exact cross-device stats),
bconv = conv3x3(pad=1) with weights sign(w)*mean(|w|) per out-channel.

Strategy: data-parallel over batch on 8 NeuronCores (16 images/core).
 - Binarized activations/weights as fp8e4 (+-1 exact); conv = 18 shifted
   matmuls (9 taps x 2 k-halves) accumulating into PSUM; integer-valued
   fp32 sums are exact.  alpha (mean|w|) is folded into the BN1/BN2 affine.
 - Exact BN via 4 tiny AllReduces of per-channel (sum, sumsq).
 - All per-channel constant shifts that are absorbed by a downstream BN
   (zeta, beta*d, d3 in the shortcut) are dropped.
 - h' (pre-BN3 activations, scaled by c3) round-trips through DRAM during
   conv2 to keep SBUF under budget.
"""

import sys

import numpy as np

sys.path.insert(0, "/opt/trn_rl_repo")

from contextlib import ExitStack

import concourse.bacc as bacc
import concourse.bass as bass
import concourse.mybir as mybir
import concourse.tile as tile
from concourse.masks import make_identity

dt = mybir.dt
AF = mybir.ActivationFunctionType
ALU = mybir.AluOpType
AX = mybir.AxisListType

C = 256
H = W = 28
PH = PW = 30
SP = PH * PW          # padded pixels / image
HW = H * W            # valid pixels / image
MARG = 32             # margin around the padded free axis (shifts up to +-31)
EPS = 1e-5
NPAR = 12
PJ = dict(g1=0, b1=1, g2=2, b2=3, g3=4, b3=5, g4=6, b4=7,
          gamma1=8, beta1=9, gamma2=10, beta2=11)


def _off(d):
    kh, kw = d // 3, d % 3
    return (kh - 1) * PW + (kw - 1)


def build_nc(n_img, n_cores):
    nc = bacc.Bacc("TRN2", target_bir_lowering=False, num_devices=n_cores,
                   name="basicblock")
    x_d = nc.declare_dram_parameter("x", [n_img, C, H, W], dt.float32, isOutput=False)
    w1_d = nc.declare_dram_parameter("w1", [C, C, 3, 3], dt.float32, isOutput=False)
    w2_d = nc.declare_dram_parameter("w2", [C, C, 3, 3], dt.float32, isOutput=False)
    p_d = nc.declare_dram_parameter("pars", [NPAR, C], dt.float32, isOutput=False)
    o_d = nc.declare_dram_parameter("out", [n_img, C, H, W], dt.float32, isOutput=True)

    FREE = n_img * SP
    XBW = FREE + 2 * MARG
    NLOC = float(n_img * HW)
    NTOT = float(n_cores * n_img * HW)
    rg = [list(range(n_cores))]

    with ExitStack() as ctx:
        tc = ctx.enter_context(tile.TileContext(nc))
        sing = ctx.enter_context(tc.tile_pool(name="sing", bufs=1))
        xbp = ctx.enter_context(tc.tile_pool(name="xbp", bufs=2))
        wtp = ctx.enter_context(tc.tile_pool(name="wtp", bufs=4))
        wop = ctx.enter_context(tc.tile_pool(name="wop", bufs=2))
        actp = ctx.enter_context(tc.tile_pool(name="actp", bufs=2 * n_img))
        chkp = ctx.enter_context(tc.tile_pool(name="chkp", bufs=4))
        tmpp = ctx.enter_context(tc.tile_pool(name="tmpp", bufs=3))
        stp = ctx.enter_context(tc.tile_pool(name="stp", bufs=1))
        psp = ctx.enter_context(tc.tile_pool(name="psp", bufs=8, space="PSUM"))
        dccp = ctx.enter_context(tc.tile_pool(name="dccp", bufs=1, space="DRAM"))
        dswp = ctx.enter_context(tc.tile_pool(name="dswp", bufs=2 * n_img, space="DRAM"))

        # ---- constants / params -------------------------------------------------
        ident = sing.tile([128, 128], dt.bfloat16, name="ident")
        make_identity(nc, ident)
        par = sing.tile([128, NPAR, 2], dt.float32, name="par")
        nc.sync.dma_start(out=par, in_=p_d[:, :].rearrange("j (h c) -> c j h", h=2))
        trash = sing.tile([128, HW], dt.float32, name="trash")
        epst = sing.tile([128, 1], dt.float32, name="epst")
        nc.vector.memset(epst, EPS)

        def P(j, ch=None):
            if ch is None:
                return par[:, PJ[j], :]
            return par[:, PJ[j], ch:ch + 1]

        # ---- persistent big buffers --------------------------------------------
        xb = [xbp.tile([128, XBW], dt.float8e4, name=f"xb{k}", tag="xb")
              for k in (0, 1)]
        for k in (0, 1):
            nc.vector.memset(xb[k], 0.0)

        wt = {}
        for cv in (1, 2):
            for k in (0, 1):
                wt[(cv, k)] = wtp.tile([128, 9, C], dt.float8e4,
                                       name=f"wt{cv}{k}", tag="wt")

        def cf(name):
            return stp.tile([128, 2], dt.float32, name=name, tag=name)

        # ---- phase 0a: x -> sign(x) into padded fp8 buffer ---------------------
        for ch in (0, 1):
            for im in range(n_img):
                xc = chkp.tile([128, HW], dt.float32, name=f"sx{ch}_{im}", tag="chk")
                nc.sync.dma_start(
                    out=xc,
                    in_=x_d[im, ch * 128:(ch + 1) * 128].rearrange("c h w -> c (h w)"))
                base = MARG + im * SP
                dst = (xb[ch][:, base:base + SP]
                       .rearrange("p (h w) -> p h w", w=PW)[:, 1:29, 1:29])
                nc.scalar.activation(dst, xc.rearrange("p (h w) -> p h w", w=W),
                                     AF.Sign)

        # ---- phase 0b: weight prep (both convs) --------------------------------
        alpha = {1: cf("alpha1"), 2: cf("alpha2")}

        def prep_w(cv, w_d):
            al = alpha[cv]
            for oh in (0, 1):
                wo = wop.tile([128, 2304], dt.bfloat16, name=f"wo{cv}{oh}", tag="wo")
                nc.gpsimd.dma_start(
                    out=wo,
                    in_=w_d[oh * 128:(oh + 1) * 128].rearrange("o i kh kw -> o (i kh kw)"))
                nc.vector.tensor_reduce(al[:, oh:oh + 1], wo, axis=AX.X, op=ALU.add,
                                        apply_absolute_value=True)
                nc.scalar.activation(wo, wo, AF.Sign)
                wos = wo.rearrange("o (i k) -> o i k", k=9)
                for ih in (0, 1):
                    for k9 in range(9):
                        pt = psp.tile([128, 128], dt.bfloat16,
                                      name=f"tp{cv}{oh}{ih}{k9}", tag="ps")
                        nc.tensor.transpose(pt, wos[:, ih * 128:(ih + 1) * 128, k9],
                                            ident)
                        nc.scalar.copy(wt[(cv, ih)][:, k9, oh * 128:(oh + 1) * 128],
                                       pt)
            nc.vector.tensor_scalar_mul(al, al, 1.0 / 2304.0)

        prep_w(1, w1_d)
        prep_w(2, w2_d)

        # ---- conv macro ---------------------------------------------------------
        def conv(cv, S, st):
            for m in (0, 1):
                for im in range(n_img):
                    s_t = actp.tile([128, HW], dt.float32,
                                    name=f"S{cv}_{m}_{im}", tag="act")
                    S[(m, im)] = s_t
                    for b in (0, 1):
                        pt = psp.tile([128, 450], dt.float32,
                                      name=f"cp{cv}_{m}_{im}_{b}", tag="ps")
                        base = MARG + im * SP + b * 450
                        n_mm = 0
                        for ki in (0, 1):
                            for d in range(9):
                                o = base + _off(d)
                                nc.tensor.matmul(
                                    pt,
                                    wt[(cv, ki)][:, d, m * 128:(m + 1) * 128],
                                    xb[ki][:, o:o + 450],
                                    start=(n_mm == 0), stop=(n_mm == 17))
                                n_mm += 1
                        pv = pt.rearrange("p (r c) -> p r c", c=PW)
                        sv = s_t.rearrange("p (r c) -> p r c", c=W)
                        r0 = 1 - b
                        nc.scalar.copy(sv[:, b * 14:(b + 1) * 14, :],
                                       pv[:, r0:r0 + 14, 1:29])
                    for q in (0, 1):
                        nc.vector.bn_stats(st[m][:, im, q],
                                           s_t[:, q * 392:(q + 1) * 392])

        # ---- stat helpers -------------------------------------------------------
        def allreduce4(s4, tag):
            di = dccp.tile([512], dt.float32, name=f"di{tag}", tag=f"di{tag}")
            do = dccp.tile([512], dt.float32, name=f"do{tag}", tag=f"do{tag}")
            nc.sync.dma_start(out=di.rearrange("(c f) -> c f", f=4), in_=s4)
            nc.gpsimd.collective_compute(
                "AllReduce", ALU.add, replica_groups=rg, ins=[di], outs=[do])
            g4 = stp.tile([128, 4], dt.float32, name=f"g4{tag}", tag=f"g4{tag}")
            nc.sync.dma_start(out=g4, in_=do.rearrange("(c f) -> c f", f=4))
            return g4

        def conv_stats_ar(st, tag):
            # st[m]: [128, n_img, 2, 6] bn_stats rows -> (sum, sumsq) cols of s4
            s4 = stp.tile([128, 4], dt.float32, name=f"s4{tag}", tag=f"s4{tag}")
            for m in (0, 1):
                mv = stp.tile([128, 2], dt.float32, name=f"mv{tag}{m}",
                              tag=f"mv{tag}{m}")
                nc.vector.bn_aggr(mv, st[m].rearrange("p a b s -> p (a b) s"))
                nc.vector.tensor_scalar_mul(s4[:, m:m + 1], mv[:, 0:1], NLOC)
                t0 = stp.tile([128, 1], dt.float32, name=f"t0{tag}{m}",
                              tag=f"t0{tag}{m}")
                nc.vector.tensor_mul(t0, mv[:, 0:1], mv[:, 0:1])
                nc.vector.tensor_add(t0, t0, mv[:, 1:2])
                nc.vector.tensor_scalar_mul(s4[:, 2 + m:3 + m], t0, NLOC)
            return allreduce4(s4, tag)

        def mean_var(g4, tag):
            mean = cf(f"mean{tag}")
            var = cf(f"var{tag}")
            msq = stp.tile([128, 2], dt.float32, name=f"msq{tag}", tag=f"msq{tag}")
            nc.vector.tensor_scalar_mul(mean, g4[:, 0:2], 1.0 / NTOT)
            nc.vector.tensor_scalar_mul(var, g4[:, 2:4], 1.0 / NTOT)
            nc.vector.tensor_mul(msq, mean, mean)
            nc.vector.tensor_sub(var, var, msq)
            return mean, var

        def inv_of(var, jg, tag):
            # g / sqrt(var + eps)
            sd = stp.tile([128, 2], dt.float32, name=f"sd{tag}", tag=f"sd{tag}")
            nc.scalar.activation(sd, var, AF.Sqrt, bias=epst)
            rc = stp.tile([128, 2], dt.float32, name=f"rc{tag}", tag=f"rc{tag}")
            nc.vector.reciprocal(rc, sd)
            inv = cf(f"inv{tag}")
            nc.vector.tensor_mul(inv, rc, P(jg))
            return inv

        def bn_conv_coefs(cv, g4, jg, jb, jgam, jbet, tag):
            # y = alpha*S:  c=alpha*inv, dg=b-alpha*mean*inv-gamma, A=1-beta, B=beta*c
            mean, var = mean_var(g4, tag)
            al = alpha[cv]
            a2 = stp.tile([128, 2], dt.float32, name=f"a2{tag}", tag=f"a2{tag}")
            nc.vector.tensor_mul(a2, al, al)
            vy = stp.tile([128, 2], dt.float32, name=f"vy{tag}", tag=f"vy{tag}")
            nc.vector.tensor_mul(vy, var, a2)
            inv = inv_of(vy, jg, tag)
            c = cf(f"c{tag}")
            nc.vector.tensor_mul(c, al, inv)
            my = stp.tile([128, 2], dt.float32, name=f"my{tag}", tag=f"my{tag}")
            nc.vector.tensor_mul(my, mean, al)
            nc.vector.tensor_mul(my, my, inv)
            dg = cf(f"dg{tag}")
            nc.vector.tensor_sub(dg, P(jb), my)
            nc.vector.tensor_sub(dg, dg, P(jgam))
            A = cf(f"A{tag}")
            nc.vector.tensor_scalar(A, P(jbet), -1.0, 1.0, ALU.mult, ALU.add)
            B = cf(f"B{tag}")
            nc.vector.tensor_mul(B, P(jbet), c)
            return c, dg, A, B

        def bn_plain_coefs(g4, jg, jb, tag):
            # c = g*inv, d = b - mean*g*inv
            mean, var = mean_var(g4, tag)
            inv = inv_of(var, jg, tag)  # = g/sqrt(var+eps)
            d = cf(f"d{tag}")
            nc.vector.tensor_mul(mean, mean, inv)
            nc.vector.tensor_sub(d, P(jb), mean)
            return inv, d

        # ---- conv1 --------------------------------------------------------------
        st1 = {m: stp.tile([128, n_img, 2, 6], dt.float32, name=f"st1_{m}",
                           tag=f"st1_{m}") for m in (0, 1)}
        S1 = {}
        conv(1, S1, st1)
        g4_1 = conv_stats_ar(st1, "bn1")
        c1, d1g, A1, B1 = bn_conv_coefs(1, g4_1, "g1", "b1", "gamma1", "beta1", "bn1")

        # ---- combine1: h' = A1*relu(c1*S+d1g) + B1*S + x  (in-place into S) -----
        hsum = {ch: stp.tile([128, n_img], dt.float32, name=f"hsum{ch}",
                             tag=f"hsum{ch}") for ch in (0, 1)}
        hsq = {ch: stp.tile([128, n_img], dt.float32, name=f"hsq{ch}",
                            tag=f"hsq{ch}") for ch in (0, 1)}
        for ch in (0, 1):
            for im in range(n_img):
                s_t = S1[(ch, im)]
                xc = chkp.tile([128, HW], dt.float32, name=f"xc{ch}_{im}", tag="chk")
                nc.sync.dma_start(
                    out=xc,
                    in_=x_d[im, ch * 128:(ch + 1) * 128].rearrange("c h w -> c (h w)"))
                t = tmpp.tile([128, HW], dt.float32, name=f"t1_{ch}_{im}", tag="t")
                nc.scalar.activation(t, s_t, AF.Relu,
                                     bias=d1g[:, ch:ch + 1], scale=c1[:, ch:ch + 1])
                nc.vector.scalar_tensor_tensor(
                    out=xc, in0=s_t, scalar=B1[:, ch:ch + 1], in1=xc,
                    op0=ALU.mult, op1=ALU.add)
                nc.vector.scalar_tensor_tensor(
                    out=s_t, in0=t, scalar=A1[:, ch:ch + 1], in1=xc,
                    op0=ALU.mult, op1=ALU.add,
                    accum_out=hsum[ch][:, im:im + 1])
                nc.scalar.activation(trash, s_t, AF.Square,
                                     accum_out=hsq[ch][:, im:im + 1])

        s4h = stp.tile([128, 4], dt.float32, name="s4h", tag="s4h")
        for ch in (0, 1):
            nc.vector.reduce_sum(s4h[:, ch:ch + 1], hsum[ch], axis=AX.X)
            nc.vector.reduce_sum(s4h[:, 2 + ch:3 + ch], hsq[ch], axis=AX.X)
        g4_3 = allreduce4(s4h, "bn3")
        c3, d3 = bn_plain_coefs(g4_3, "g3", "b3", "bn3")

        # ---- BN3: scale h' by c3 in place, sign -> xb2, swap h'' to DRAM --------
        HSW = {}
        for ch in (0, 1):
            for im in range(n_img):
                s_t = S1[(ch, im)]
                nc.scalar.activation(s_t, s_t, AF.Copy, scale=c3[:, ch:ch + 1])
                base = MARG + im * SP
                dst = (xb[ch][:, base:base + SP]
                       .rearrange("p (h w) -> p h w", w=PW)[:, 1:29, 1:29])
                nc.scalar.activation(dst, s_t.rearrange("p (h w) -> p h w", w=W),
                                     AF.Sign, bias=d3[:, ch:ch + 1])
                dr = dswp.tile([128, HW], dt.float32, name=f"hs{ch}_{im}", tag="swap")
                HSW[(ch, im)] = dr
                nc.sync.dma_start(out=dr, in_=s_t)

        # ---- conv2 --------------------------------------------------------------
        st2 = {m: stp.tile([128, n_img, 2, 6], dt.float32, name=f"st2_{m}",
                           tag=f"st2_{m}") for m in (0, 1)}
        S2 = {}
        conv(2, S2, st2)
        g4_2 = conv_stats_ar(st2, "bn2")
        c2, d2g, A2, B2 = bn_conv_coefs(2, g4_2, "g2", "b2", "gamma2", "beta2", "bn2")

        # ---- combine2: y = A2*relu(c2*S2+d2g) + B2*S2 + (c3*h'+d3 - d3) ---------
        hsum2 = {ch: stp.tile([128, n_img], dt.float32, name=f"hsum2{ch}",
                              tag=f"hsum2{ch}") for ch in (0, 1)}
        hsq2 = {ch: stp.tile([128, n_img], dt.float32, name=f"hsq2{ch}",
                             tag=f"hsq2{ch}") for ch in (0, 1)}
        for ch in (0, 1):
            for im in range(n_img):
                s2 = S2[(ch, im)]
                hc = chkp.tile([128, HW], dt.float32, name=f"hc{ch}_{im}", tag="chk")
                nc.sync.dma_start(out=hc, in_=HSW[(ch, im)])
                t2 = tmpp.tile([128, HW], dt.float32, name=f"t2_{ch}_{im}", tag="t")
                nc.scalar.activation(t2, s2, AF.Relu,
                                     bias=d2g[:, ch:ch + 1], scale=c2[:, ch:ch + 1])
                nc.vector.scalar_tensor_tensor(
                    out=hc, in0=s2, scalar=B2[:, ch:ch + 1], in1=hc,
                    op0=ALU.mult, op1=ALU.add)
                nc.vector.scalar_tensor_tensor(
                    out=s2, in0=t2, scalar=A2[:, ch:ch + 1], in1=hc,
                    op0=ALU.mult, op1=ALU.add,
                    accum_out=hsum2[ch][:, im:im + 1])
                nc.scalar.activation(trash, s2, AF.Square,
                                     accum_out=hsq2[ch][:, im:im + 1])

        s4f = stp.tile([128, 4], dt.float32, name="s4f", tag="s4f")
        for ch in (0, 1):
            nc.vector.reduce_sum(s4f[:, ch:ch + 1], hsum2[ch], axis=AX.X)
            nc.vector.reduce_sum(s4f[:, 2 + ch:3 + ch], hsq2[ch], axis=AX.X)
        g4_4 = allreduce4(s4f, "bn4")
        c4, d4 = bn_plain_coefs(g4_4, "g4", "b4", "bn4")

        # ---- BN4 + output -------------------------------------------------------
        for ch in (0, 1):
            for im in range(n_img):
                s2 = S2[(ch, im)]
                nc.scalar.activation(s2, s2, AF.Identity,
                                     bias=d4[:, ch:ch + 1], scale=c4[:, ch:ch + 1])
                nc.sync.dma_start(
                    out=o_d[im, ch * 128:(ch + 1) * 128].rearrange("c h w -> c (h w)"),
                    in_=s2)

    nc.compile()
    return nc


_NC_CACHE = {}


def get_nc(n_img, n_cores):
    key = (n_img, n_cores)
    if key not in _NC_CACHE:
        _NC_CACHE[key] = build_nc(n_img, n_cores)
    return _NC_CACHE[key]


def pack_pars(inputs):
    return np.stack([np.asarray(inputs[k], np.float32) for k in
                     ["g1", "b1", "g2", "b2", "g3", "b3", "g4", "b4",
                      "gamma1", "beta1", "gamma2", "beta2"]])


def kernel(**inputs):
    from concourse.bass_utils import run_bass_kernel_spmd

    x = np.asarray(inputs["x"], np.float32)
    n_cores = 8
    n_img = x.shape[0] // n_cores
    nc = get_nc(n_img, n_cores)
    pars = pack_pars(inputs)
    w1 = np.asarray(inputs["w1"], np.float32)
    w2 = np.asarray(inputs["w2"], np.float32)
    in_maps = [
        {"x": np.ascontiguousarray(x[c * n_img:(c + 1) * n_img]),
         "w1": w1, "w2": w2, "pars": pars}
        for c in range(n_cores)
    ]
    res = run_bass_kernel_spmd(nc, in_maps, core_ids=list(range(n_cores)))
    return np.concatenate([res.results[c]["out"] for c in range(n_cores)], axis=0)


if __name__ == "__main__":
    nc = build_nc(2, 2)
    print("built ok:", len(nc.m.functions[0].blocks if hasattr(nc.m.functions[0], 'blocks') else []))


# revision 15
# speedup vs baseline: 1.3402x; 1.3402x over previous
"""Trainium2 Bass kernel for nn_BasicBlock (binary-conv residual block).

Math (reference):
  h  = BN3( RPReLU1(BN1(bconv(sign(x), w1))) + x )
  out= BN4( RPReLU2(BN2(bconv(sign(h), w2))) + h )
with training-mode BN over the FULL batch (exact cross-device stats),
bconv = conv3x3(pad=1) with weights sign(w)*mean(|w|) per out-channel.

Strategy: data-parallel over batch on 8 NeuronCores (16 images/core).
 - Binarized activations/weights as fp8e4 (+-1 exact); conv = 18 shifted
   matmuls (9 taps x 2 k-halves) accumulating into PSUM; integer-valued
   fp32 sums are exact.  alpha (mean|w|) is folded into the BN1/BN2 affine.
 - Exact BN via 4 tiny AllReduces of per-channel (sum, sumsq).
 - All per-channel constant shifts that are absorbed by a downstream BN
   (zeta, beta*d, d3 in the shortcut) are dropped.
 - h' (pre-BN3 activations, scaled by c3) round-trips through DRAM during
   conv2 to keep SBUF under budget.
"""

import sys

import numpy as np

sys.path.insert(0, "/opt/trn_rl_repo")

from contextlib import ExitStack

import concourse.bacc as bacc
import concourse.bass as bass
import concourse.bass_utils as _bu
import concourse.mybir as mybir
import concourse.tile as tile
from concourse.masks import make_identity

# Note: --enable-ldw-opt=true fails walrus codegen ("InstLdweights is not
# compatible with LDW optimization") because Bacc emits standalone LDWEIGHTS
# to carry overflow sem-waits, so we live with per-matmul weight loads.

dt = mybir.dt
AF = mybir.ActivationFunctionType
ALU = mybir.AluOpType
AX = mybir.AxisListType

C = 256
H = W = 28
PH = PW = 30
SP = PH * PW          # padded pixels / image
HW = H * W            # valid pixels / image
MARG = 32             # margin around the padded free axis (shifts up to +-31)
EPS = 1e-5
NPAR = 12
PJ = dict(g1=0, b1=1, g2=2, b2=3, g3=4, b3=5, g4=6, b4=7,
          gamma1=8, beta1=9, gamma2=10, beta2=11)


def _off(d):
    kh, kw = d // 3, d % 3
    return (kh - 1) * PW + (kw - 1)


def build_nc(n_img, n_cores):
    nc = bacc.Bacc("TRN2", target_bir_lowering=False, num_devices=n_cores,
                   name="basicblock")
    x_d = nc.declare_dram_parameter("x", [n_img, C, H, W], dt.float32, isOutput=False)
    w1_d = nc.declare_dram_parameter("w1", [C, C, 3, 3], dt.float32, isOutput=False)
    w2_d = nc.declare_dram_parameter("w2", [C, C, 3, 3], dt.float32, isOutput=False)
    p_d = nc.declare_dram_parameter("pars", [NPAR, C], dt.float32, isOutput=False)
    o_d = nc.declare_dram_parameter("out", [n_img, C, H, W], dt.float32, isOutput=True)

    FREE = n_img * SP
    XBW = FREE + 2 * MARG
    NLOC = float(n_img * HW)
    NTOT = float(n_cores * n_img * HW)
    rg = [list(range(n_cores))]

    with ExitStack() as ctx:
        tc = ctx.enter_context(tile.TileContext(nc))
        sing = ctx.enter_context(tc.tile_pool(name="sing", bufs=1))
        xbp = ctx.enter_context(tc.tile_pool(name="xbp", bufs=1))
        wtp = ctx.enter_context(tc.tile_pool(name="wtp", bufs=2))
        wop = ctx.enter_context(tc.tile_pool(name="wop", bufs=2))
        actp = ctx.enter_context(tc.tile_pool(name="actp", bufs=2 * n_img))
        chkp = ctx.enter_context(tc.tile_pool(name="chkp", bufs=4))
        tmpp = ctx.enter_context(tc.tile_pool(name="tmpp", bufs=3))
        stp = ctx.enter_context(tc.tile_pool(name="stp", bufs=1))
        psp = ctx.enter_context(tc.tile_pool(name="psp", bufs=8, space="PSUM"))
        dccp = ctx.enter_context(tc.tile_pool(name="dccp", bufs=1, space="DRAM"))
        dswp = ctx.enter_context(tc.tile_pool(name="dswp", bufs=2 * n_img, space="DRAM"))

        # ---- constants / params -------------------------------------------------
        ident = sing.tile([128, 128], dt.bfloat16, name="ident")
        make_identity(nc, ident)
        par = sing.tile([128, NPAR, 2], dt.float32, name="par")
        nc.sync.dma_start(out=par, in_=p_d[:, :].rearrange("j (h c) -> c j h", h=2))
        trash = sing.tile([128, HW], dt.float32, name="trash")
        epst = sing.tile([128, 1], dt.float32, name="epst")
        nc.vector.memset(epst, EPS)

        def P(j, ch=None):
            if ch is None:
                return par[:, PJ[j], :]
            return par[:, PJ[j], ch:ch + 1]

        # ---- persistent big buffers --------------------------------------------
        # xb: [128, 2(k-half), XBW] fp8, DoubleRow-interleaved conv input
        xbt = xbp.tile([128, 2, XBW], dt.float8e4, name="xbt", tag="xb")
        nc.vector.memset(xbt, 0.0)

        # wt: [128(i), 2(k-half), 9(tap), 256(o)] fp8 per conv
        wt = {cv: wtp.tile([128, 2, 9, C], dt.float8e4, name=f"wt{cv}", tag="wt")
              for cv in (1, 2)}

        def cf(name):
            return stp.tile([128, 2], dt.float32, name=name, tag=name)

        # ---- phase 0a: x -> sign(x) into padded fp8 buffer ---------------------
        for ch in (0, 1):
            for im in range(n_img):
                xc = chkp.tile([128, HW], dt.float32, name=f"sx{ch}_{im}", tag="chk")
                nc.sync.dma_start(
                    out=xc,
                    in_=x_d[im, ch * 128:(ch + 1) * 128].rearrange("c h w -> c (h w)"))
                base = MARG + im * SP
                dst = (xbt[:, ch, base:base + SP]
                       .rearrange("p (h w) -> p h w", w=PW)[:, 1:29, 1:29])
                nc.scalar.activation(dst, xc.rearrange("p (h w) -> p h w", w=W),
                                     AF.Sign)

        # ---- phase 0b: weight prep (both convs) --------------------------------
        alpha = {1: cf("alpha1"), 2: cf("alpha2")}

        def prep_w(cv, w_d):
            al = alpha[cv]
            for oh in (0, 1):
                wo = wop.tile([128, 2304], dt.bfloat16, name=f"wo{cv}{oh}", tag="wo")
                nc.gpsimd.dma_start(
                    out=wo,
                    in_=w_d[oh * 128:(oh + 1) * 128].rearrange("o i kh kw -> o (i kh kw)"))
                nc.vector.tensor_reduce(al[:, oh:oh + 1], wo, axis=AX.X, op=ALU.add,
                                        apply_absolute_value=True)
                nc.scalar.activation(wo, wo, AF.Sign)
                wos = wo.rearrange("o (i k) -> o i k", k=9)
                for ih in (0, 1):
                    for k9 in range(9):
                        pt = psp.tile([128, 128], dt.bfloat16,
                                      name=f"tp{cv}{oh}{ih}{k9}", tag="ps")
                        nc.tensor.transpose(pt, wos[:, ih * 128:(ih + 1) * 128, k9],
                                            ident)
                        nc.scalar.copy(wt[cv][:, ih, k9, oh * 128:(oh + 1) * 128],
                                       pt)
            nc.vector.tensor_scalar_mul(al, al, 1.0 / 2304.0)

        prep_w(1, w1_d)
        prep_w(2, w2_d)

        # ---- conv macro ---------------------------------------------------------
        # DoubleRow fp8: one matmul contracts both 128-channel halves.
        # Weight-stationary: each (m, tap) weight serves a group of 8 psum
        # banks before switching (redundant LDWEIGHTS elided by ldw-opt).
        def conv(cv, S, st):
            tiles = [(im, b) for im in range(n_img) for b in (0, 1)]
            for m in (0, 1):
                for im in range(n_img):
                    S[(m, im)] = actp.tile([128, HW], dt.float32,
                                           name=f"S{cv}_{m}_{im}", tag="act")
                for g0 in range(0, len(tiles), 8):
                    grp = tiles[g0:g0 + 8]
                    pts = {}
                    for (im, b) in grp:
                        pts[(im, b)] = psp.tile([128, 450], dt.float32,
                                                name=f"cp{cv}_{m}_{im}_{b}",
                                                tag="ps")
                    for d in range(9):
                        w_ap = wt[cv][:, :, d, m * 128:(m + 1) * 128]
                        for (im, b) in grp:
                            o = MARG + im * SP + b * 450 + _off(d)
                            nc.tensor.matmul(
                                pts[(im, b)], w_ap, xbt[:, :, o:o + 450],
                                perf_mode=mybir.MatmulPerfMode.DoubleRow,
                                start=(d == 0), stop=(d == 8))
                    for (im, b) in grp:
                        pt = pts[(im, b)]
                        s_t = S[(m, im)]
                        pv = pt.rearrange("p (r c) -> p r c", c=PW)
                        sv = s_t.rearrange("p (r c) -> p r c", c=W)
                        r0 = 1 - b
                        nc.scalar.copy(sv[:, b * 14:(b + 1) * 14, :],
                                       pv[:, r0:r0 + 14, 1:29])
                        if b == 1:
                            for q in (0, 1):
                                nc.vector.bn_stats(st[m][:, im, q],
                                                   s_t[:, q * 392:(q + 1) * 392])

        # ---- stat helpers -------------------------------------------------------
        def allreduce4(s4, tag):
            di = dccp.tile([512], dt.float32, name=f"di{tag}", tag=f"di{tag}")
            do = dccp.tile([512], dt.float32, name=f"do{tag}", tag=f"do{tag}")
            nc.sync.dma_start(out=di.rearrange("(c f) -> c f", f=4), in_=s4)
            nc.gpsimd.collective_compute(
                "AllReduce", ALU.add, replica_groups=rg, ins=[di], outs=[do])
            g4 = stp.tile([128, 4], dt.float32, name=f"g4{tag}", tag=f"g4{tag}")
            nc.sync.dma_start(out=g4, in_=do.rearrange("(c f) -> c f", f=4))
            return g4

        def conv_stats_ar(st, tag):
            # st[m]: [128, n_img, 2, 6] bn_stats rows -> (sum, sumsq) cols of s4
            s4 = stp.tile([128, 4], dt.float32, name=f"s4{tag}", tag=f"s4{tag}")
            for m in (0, 1):
                mv = stp.tile([128, 2], dt.float32, name=f"mv{tag}{m}",
                              tag=f"mv{tag}{m}")
                nc.vector.bn_aggr(mv, st[m].rearrange("p a b s -> p (a b) s"))
                nc.vector.tensor_scalar_mul(s4[:, m:m + 1], mv[:, 0:1], NLOC)
                t0 = stp.tile([128, 1], dt.float32, name=f"t0{tag}{m}",
                              tag=f"t0{tag}{m}")
                nc.vector.tensor_mul(t0, mv[:, 0:1], mv[:, 0:1])
                nc.vector.tensor_add(t0, t0, mv[:, 1:2])
                nc.vector.tensor_scalar_mul(s4[:, 2 + m:3 + m], t0, NLOC)
            return allreduce4(s4, tag)

        def mean_var(g4, tag):
            mean = cf(f"mean{tag}")
            var = cf(f"var{tag}")
            msq = stp.tile([128, 2], dt.float32, name=f"msq{tag}", tag=f"msq{tag}")
            nc.vector.tensor_scalar_mul(mean, g4[:, 0:2], 1.0 / NTOT)
            nc.vector.tensor_scalar_mul(var, g4[:, 2:4], 1.0 / NTOT)
            nc.vector.tensor_mul(msq, mean, mean)
            nc.vector.tensor_sub(var, var, msq)
            return mean, var

        def inv_of(var, jg, tag):
            # g / sqrt(var + eps)
            sd = stp.tile([128, 2], dt.float32, name=f"sd{tag}", tag=f"sd{tag}")
            nc.scalar.activation(sd, var, AF.Sqrt, bias=epst)
            rc = stp.tile([128, 2], dt.float32, name=f"rc{tag}", tag=f"rc{tag}")
            nc.vector.reciprocal(rc, sd)
            inv = cf(f"inv{tag}")
            nc.vector.tensor_mul(inv, rc, P(jg))
            return inv

        def bn_conv_coefs(cv, g4, jg, jb, jgam, jbet, tag):
            # y = alpha*S:  c=alpha*inv, dg=b-alpha*mean*inv-gamma, A=1-beta, B=beta*c
            mean, var = mean_var(g4, tag)
            al = alpha[cv]
            a2 = stp.tile([128, 2], dt.float32, name=f"a2{tag}", tag=f"a2{tag}")
            nc.vector.tensor_mul(a2, al, al)
            vy = stp.tile([128, 2], dt.float32, name=f"vy{tag}", tag=f"vy{tag}")
            nc.vector.tensor_mul(vy, var, a2)
            inv = inv_of(vy, jg, tag)
            c = cf(f"c{tag}")
            nc.vector.tensor_mul(c, al, inv)
            my = stp.tile([128, 2], dt.float32, name=f"my{tag}", tag=f"my{tag}")
            nc.vector.tensor_mul(my, mean, al)
            nc.vector.tensor_mul(my, my, inv)
            dg = cf(f"dg{tag}")
            nc.vector.tensor_sub(dg, P(jb), my)
            nc.vector.tensor_sub(dg, dg, P(jgam))
            A = cf(f"A{tag}")
            nc.vector.tensor_scalar(A, P(jbet), -1.0, 1.0, ALU.mult, ALU.add)
            B = cf(f"B{tag}")
            nc.vector.tensor_mul(B, P(jbet), c)
            return c, dg, A, B

        def bn_plain_coefs(g4, jg, jb, tag):
            # c = g*inv, d = b - mean*g*inv
            mean, var = mean_var(g4, tag)
            inv = inv_of(var, jg, tag)  # = g/sqrt(var+eps)
            d = cf(f"d{tag}")
            nc.vector.tensor_mul(mean, mean, inv)
            nc.vector.tensor_sub(d, P(jb), mean)
            return inv, d

        # ---- conv1 --------------------------------------------------------------
        st1 = {m: stp.tile([128, n_img, 2, 6], dt.float32, name=f"st1_{m}",
                           tag=f"st1_{m}") for m in (0, 1)}
        S1 = {}
        conv(1, S1, st1)
        g4_1 = conv_stats_ar(st1, "bn1")
        c1, d1g, A1, B1 = bn_conv_coefs(1, g4_1, "g1", "b1", "gamma1", "beta1", "bn1")

        # ---- combine1: h' = A1*relu(c1*S+d1g) + B1*S + x  (in-place into S) -----
        hsum = {ch: stp.tile([128, n_img], dt.float32, name=f"hsum{ch}",
                             tag=f"hsum{ch}") for ch in (0, 1)}
        hsq = {ch: stp.tile([128, n_img], dt.float32, name=f"hsq{ch}",
                            tag=f"hsq{ch}") for ch in (0, 1)}
        for ch in (0, 1):
            for im in range(n_img):
                s_t = S1[(ch, im)]
                xc = chkp.tile([128, HW], dt.float32, name=f"xc{ch}_{im}", tag="chk")
                nc.sync.dma_start(
                    out=xc,
                    in_=x_d[im, ch * 128:(ch + 1) * 128].rearrange("c h w -> c (h w)"))
                t = tmpp.tile([128, HW], dt.float32, name=f"t1_{ch}_{im}", tag="t")
                nc.scalar.activation(t, s_t, AF.Relu,
                                     bias=d1g[:, ch:ch + 1], scale=c1[:, ch:ch + 1])
                nc.vector.scalar_tensor_tensor(
                    out=xc, in0=s_t, scalar=B1[:, ch:ch + 1], in1=xc,
                    op0=ALU.mult, op1=ALU.add)
                nc.vector.scalar_tensor_tensor(
                    out=s_t, in0=t, scalar=A1[:, ch:ch + 1], in1=xc,
                    op0=ALU.mult, op1=ALU.add,
                    accum_out=hsum[ch][:, im:im + 1])
                nc.scalar.activation(trash, s_t, AF.Square,
                                     accum_out=hsq[ch][:, im:im + 1])

        s4h = stp.tile([128, 4], dt.float32, name="s4h", tag="s4h")
        for ch in (0, 1):
            nc.vector.reduce_sum(s4h[:, ch:ch + 1], hsum[ch], axis=AX.X)
            nc.vector.reduce_sum(s4h[:, 2 + ch:3 + ch], hsq[ch], axis=AX.X)
        g4_3 = allreduce4(s4h, "bn3")
        c3, d3 = bn_plain_coefs(g4_3, "g3", "b3", "bn3")

        # ---- BN3: scale h' by c3 in place, sign -> xb2, swap h'' to DRAM --------
        HSW = {}
        for ch in (0, 1):
            for im in range(n_img):
                s_t = S1[(ch, im)]
                nc.scalar.activation(s_t, s_t, AF.Copy, scale=c3[:, ch:ch + 1])
                base = MARG + im * SP
                dst = (xbt[:, ch, base:base + SP]
                       .rearrange("p (h w) -> p h w", w=PW)[:, 1:29, 1:29])
                nc.scalar.activation(dst, s_t.rearrange("p (h w) -> p h w", w=W),
                                     AF.Sign, bias=d3[:, ch:ch + 1])
                dr = dswp.tile([128, HW], dt.float32, name=f"hs{ch}_{im}", tag="swap")
                HSW[(ch, im)] = dr
                nc.sync.dma_start(out=dr, in_=s_t)

        # ---- conv2 --------------------------------------------------------------
        st2 = {m: stp.tile([128, n_img, 2, 6], dt.float32, name=f"st2_{m}",
                           tag=f"st2_{m}") for m in (0, 1)}
        S2 = {}
        conv(2, S2, st2)
        g4_2 = conv_stats_ar(st2, "bn2")
        c2, d2g, A2, B2 = bn_conv_coefs(2, g4_2, "g2", "b2", "gamma2", "beta2", "bn2")

        # ---- combine2: y = A2*relu(c2*S2+d2g) + B2*S2 + (c3*h'+d3 - d3) ---------
        hsum2 = {ch: stp.tile([128, n_img], dt.float32, name=f"hsum2{ch}",
                              tag=f"hsum2{ch}") for ch in (0, 1)}
        hsq2 = {ch: stp.tile([128, n_img], dt.float32, name=f"hsq2{ch}",
                             tag=f"hsq2{ch}") for ch in (0, 1)}
        for ch in (0, 1):
            for im in range(n_img):
                s2 = S2[(ch, im)]
                hc = chkp.tile([128, HW], dt.float32, name=f"hc{ch}_{im}", tag="chk")
                nc.sync.dma_start(out=hc, in_=HSW[(ch, im)])
                t2 = tmpp.tile([128, HW], dt.float32, name=f"t2_{ch}_{im}", tag="t")
                nc.scalar.activation(t2, s2, AF.Relu,
                                     bias=d2g[:, ch:ch + 1], scale=c2[:, ch:ch + 1])
                nc.vector.scalar_tensor_tensor(
                    out=hc, in0=s2, scalar=B2[:, ch:ch + 1], in1=hc,
                    op0=ALU.mult, op1=ALU.add)
                nc.vector.scalar_tensor_tensor(
                    out=s2, in0=t2, scalar=A2[:, ch:ch + 1], in1=hc,
                    op0=ALU.mult, op1=ALU.add,
                    accum_out=hsum2[ch][:, im:im + 1])
                nc.scalar.activation(trash, s2, AF.Square,
                                     accum_out=hsq2[ch][:, im:im + 1])

        s4f = stp.tile([128, 4], dt.float32, name="s4f", tag="s4f")
        for ch in (0, 1):
            nc.vector.reduce_sum(s4f[:, ch:ch + 1], hsum2[ch], axis=AX.X)
            nc.vector.reduce_sum(s4f[:, 2 + ch:3 + ch], hsq2[ch], axis=AX.X)
        g4_4 = allreduce4(s4f, "bn4")
        c4, d4 = bn_plain_coefs(g4_4, "g4", "b4", "bn4")

        # ---- BN4 + output -------------------------------------------------------
        for ch in (0, 1):
            for im in range(n_img):
                s2 = S2[(ch, im)]
                nc.scalar.activation(s2, s2, AF.Identity,
                                     bias=d4[:, ch:ch + 1], scale=c4[:, ch:ch + 1])
                nc.sync.dma_start(
                    out=o_d[im, ch * 128:(ch + 1) * 128].rearrange("c h w -> c (h w)"),
                    in_=s2)

    nc.compile()
    return nc


_NC_CACHE = {}


def get_nc(n_img, n_cores):
    key = (n_img, n_cores)
    if key not in _NC_CACHE:
        _NC_CACHE[key] = build_nc(n_img, n_cores)
    return _NC_CACHE[key]


def pack_pars(inputs):
    return np.stack([np.asarray(inputs[k], np.float32) for k in
                     ["g1", "b1", "g2", "b2", "g3", "b3", "g4", "b4",
                      "gamma1", "beta1", "gamma2", "beta2"]])


def kernel(**inputs):
    from concourse.bass_utils import run_bass_kernel_spmd

    x = np.asarray(inputs["x"], np.float32)
    n_cores = 8
    n_img = x.shape[0] // n_cores
    nc = get_nc(n_img, n_cores)
    pars = pack_pars(inputs)
    w1 = np.asarray(inputs["w1"], np.float32)
    w2 = np.asarray(inputs["w2"], np.float32)
    in_maps = [
        {"x": np.ascontiguousarray(x[c * n_img:(c + 1) * n_img]),
         "w1": w1, "w2": w2, "pars": pars}
        for c in range(n_cores)
    ]
    res = run_bass_kernel_spmd(nc, in_maps, core_ids=list(range(n_cores)))
    return np.concatenate([res.results[c]["out"] for c in range(n_cores)], axis=0)


if __name__ == "__main__":
    nc = build_nc(2, 2)
    print("built ok:", len(nc.m.functions[0].blocks if hasattr(nc.m.functions[0], 'blocks') else []))
